# revision 17
# baseline (speedup 1.0000x reference)
import sys

if "/opt/trn_rl_repo" not in sys.path:
    sys.path.insert(0, "/opt/trn_rl_repo")

import numpy as np

B, HD, H, W, K = 2, 4, 128, 128, 49
KS = 7
NSP = 9
S = 64
N_CORES = 8
WQ = W // 4            # 32 columns per core
TPG = 1                # tiles (columns) per dma_gather group
NGRP = WQ // TPG       # 8 groups
NI_T = H * NSP         # 1152 indices per tile
NI_G = NI_T * TPG      # 4608 per group
ROWS_T = S * H         # 8192 band rows per tile
ELEM = 64              # 256B gather unit: 49 patch + pi + pad
EPS = 1e-10

_cached = {}


def _build():
    import concourse.bass as bass
    import concourse.tile as tile
    from concourse import bacc, mybir

    f32 = mybir.dt.float32
    i16 = mybir.dt.int16

    nc = bacc.Bacc("TRN2", target_bir_lowering=False, debug=False, num_devices=N_CORES)
    attn_s = nc.dram_tensor("attn", [HD, H, WQ, K], f32, kind="ExternalInput")
    band = nc.dram_tensor("band", [WQ, ROWS_T, ELEM], f32, kind="ExternalInput")
    idxw = nc.dram_tensor("idxw", [128, NGRP * (NI_G // 16)], i16, kind="ExternalInput")
    out_s = nc.dram_tensor("out", [HD, H, WQ, K], f32, kind="ExternalOutput")

    HD_K = HD * K          # 196
    U_SZ = HD * NSP * K    # 1764

    def ap(t, off, dims):
        return bass.AP(t, off, [list(d) for d in dims])

    def sap(tap, extra_off, dims):
        # SBUF AP: keep the partition dim, replace free dims
        return bass.AP(tap.tensor, tap.offset + extra_off, [list(tap.ap[0]), *[list(d) for d in dims]])

    at_h, at_hd, at_w = WQ * K * H, 0, 0  # placeholder
    # attn/out DRAM strides (hd, h, wl, k) contiguous
    s_hd, s_h, s_w = H * WQ * K, WQ * K, K

    with tile.TileContext(nc) as tc:
        with (
            tc.tile_pool(name="idxp", bufs=4) as idxp,
            tc.tile_pool(name="gp", bufs=8) as gp,
            tc.tile_pool(name="inp", bufs=6) as inp,
            tc.tile_pool(name="up", bufs=3) as up,
            tc.tile_pool(name="sp", bufs=4) as sp,
            tc.tile_pool(name="op", bufs=4) as op,
        ):
            idx_t = idxp.tile([128, NGRP * (NI_G // 16)], i16)
            nc.sync.dma_start(idx_t[:], idxw.ap())

            for g in range(NGRP):
                G = gp.tile([128, (NI_G // 128) * ELEM], f32, tag="G")
                nc.gpsimd.dma_gather(
                    out_ap=G[:].rearrange("p (a b) -> p a b", b=ELEM),
                    in_ap=ap(band, g * TPG * ROWS_T * ELEM, [(ELEM, TPG * ROWS_T), (1, ELEM)]),
                    idxs_ap=idx_t[:, g * (NI_G // 16):(g + 1) * (NI_G // 16)],
                    num_idxs=NI_G,
                    num_idxs_reg=NI_G,
                    elem_size=ELEM,
                    single_packet=False,
                )
                for t in range(TPG):
                    wl = g * TPG + t
                    gb = t * NSP * ELEM

                    at4 = inp.tile([128, HD_K], f32, tag="at")
                    nc.sync.dma_start(
                        at4[:],
                        ap(attn_s, wl * s_w, [(s_h, H), (s_hd, HD), (1, K)]),
                    )
                    e4 = inp.tile([128, HD_K], f32, tag="e")
                    nc.scalar.activation(e4[:], at4[:], mybir.ActivationFunctionType.Exp)

                    # u4[hd,s,ji] = G.patch[s,ji] * e4[hd,ji]
                    u4 = up.tile([128, U_SZ], f32, tag="u")
                    nc.vector.tensor_tensor(
                        out=u4[:].rearrange("p (a b c) -> p a b c", a=HD, b=NSP),
                        in0=sap(G[:], gb, [(0, HD), (ELEM, NSP), (1, K)]),
                        in1=sap(e4[:], 0, [(K, HD), (0, NSP), (1, K)]),
                        op=mybir.AluOpType.mult,
                    )
                    # d4[hd,s] = sum_ji u4
                    d4 = sp.tile([128, HD * NSP], f32, tag="d")
                    nc.vector.reduce_sum(
                        out=d4[:].rearrange("p (a b) -> p a b", a=HD),
                        in_=u4[:].rearrange("p (a b c) -> p a b c", a=HD, b=NSP),
                        axis=mybir.AxisListType.X,
                    )
                    r4 = sp.tile([128, HD * NSP], f32, tag="r")
                    nc.vector.reciprocal_approx_fast(out=r4[:], in_=d4[:])
                    # w4[hd,s] = r4 * pi[s]
                    w4 = sp.tile([128, HD * NSP], f32, tag="w")
                    nc.vector.tensor_tensor(
                        out=w4[:].rearrange("p (a b) -> p a b", a=HD),
                        in0=r4[:].rearrange("p (a b) -> p a b", a=HD),
                        in1=sap(G[:], gb + K, [(0, HD), (ELEM, NSP)]),
                        op=mybir.AluOpType.mult,
                    )
                    # acc4[hd,s,ji] = u4 * w4 (broadcast over ji)
                    acc4 = up.tile([128, U_SZ], f32, tag="acc")
                    nc.vector.tensor_tensor(
                        out=acc4[:].rearrange("p (a b) -> p a b", a=HD * NSP),
                        in0=u4[:].rearrange("p (a b) -> p a b", a=HD * NSP),
                        in1=sap(w4[:], 0, [(1, HD * NSP), (0, K)]),
                        op=mybir.AluOpType.mult,
                    )
                    # o4[hd,ji] = sum_s acc4   (strided in, s innermost)
                    o4 = op.tile([128, HD_K], f32, tag="o")
                    nc.vector.reduce_sum(
                        out=o4[:].rearrange("p (a b) -> p a b", a=HD),
                        in_=sap(acc4[:], 0, [(NSP * K, HD), (1, K), (K, NSP)]),
                        axis=mybir.AxisListType.X,
                    )
                    nc.sync.dma_start(
                        ap(out_s, wl * s_w, [(s_h, H), (s_hd, HD), (1, K)]),
                        o4[:],
                    )
    nc.compile()
    return nc


def _host_prep(attn, sims, sinds):
    hj = (np.clip(np.arange(H) - KS // 2, 0, H - KS)[:, None] + np.arange(KS)[None, :])
    wj = (np.clip(np.arange(W) - KS // 2, 0, W - KS)[:, None] + np.arange(KS)[None, :])
    harange = np.arange(H, dtype=np.int32)
    in_maps = []
    for b in range(B):
        sims_b = sims[b]                                  # (S,H,W)
        SW = np.ascontiguousarray(sims_b[:, hj, :])       # (S,H,7,W)
        for q in range(4):
            wsl = slice(WQ * q, WQ * (q + 1))
            attn_b = np.ascontiguousarray(attn[b][:, :, wsl, :])  # (HD,H,WQ,K)
            wq = wj[wsl]                                  # (WQ,7)
            band = np.zeros((WQ, S, H, ELEM), dtype=np.float32)
            # band[w0l,c,h,j*7+i] = SW[c,h,j,wq[w0l,i]]
            band[..., :K] = SW[:, :, :, wq].transpose(3, 0, 1, 2, 4).reshape(WQ, S, H, K)
            band[..., K] = sims_b[:, :, wsl].transpose(2, 0, 1)   # pi = sims[c,h,w0]
            band = band.reshape(WQ, ROWS_T, ELEM)

            g = sinds[b][:, wsl, :].astype(np.int32)      # (H,WQ,NSP)
            idxv = g * H + harange[:, None, None]         # (H,WQ,NSP)
            arr = idxv.transpose(1, 2, 0)                 # (WQ,NSP,H)
            grp = arr.reshape(NGRP, TPG, NSP, H) + (np.arange(TPG, dtype=np.int32) * ROWS_T)[None, :, None, None]
            lst = grp.reshape(NGRP, NI_G).astype(np.int16)
            wr = lst.reshape(NGRP, NI_G // 16, 16).transpose(0, 2, 1)
            idxw = np.tile(wr, (1, 8, 1)).transpose(1, 0, 2).reshape(128, NGRP * (NI_G // 16))
            in_maps.append({
                "attn": attn_b,
                "band": band,
                "idxw": np.ascontiguousarray(idxw),
            })
    return in_maps


def kernel(attn, sims, sinds):
    from concourse.bass_utils import run_bass_kernel_spmd

    attn = np.asarray(attn, dtype=np.float32)
    sims = np.asarray(sims, dtype=np.float32)
    sinds = np.asarray(sinds)

    if "nc" not in _cached:
        _cached["nc"] = _build()
    nc = _cached["nc"]

    in_maps = _host_prep(attn, sims, sinds)
    res = run_bass_kernel_spmd(nc, in_maps, list(range(N_CORES)))

    out = np.empty((B, HD, H, W, K), dtype=np.float32)
    for cid in range(N_CORES):
        b, q = divmod(cid, 4)
        out[b][:, :, WQ * q:WQ * (q + 1), :] = res.results[cid]["out"]
    return out
